# revision 6
# baseline (speedup 1.0000x reference)
"""TRN2 Bass kernel for the quantized 4-layer MLP (dense_mlp, 8 cores).

Strategy:
  - Data-parallel over batch: each of the 8 cores gets 1024 of 8192 rows.
  - Activations kept transposed [feature, batch] on-chip the whole way;
    quantized activation LEVELS (ints 0..15) stored as fp8e4 (exact).
  - Layer 1: x split into fp16 hi+lo parts (2 fp16 matmuls; 22+ mantissa
    bits total => fp32-level accuracy), weights quantized to ints -3..3
    (exact in fp16).
  - Layers 2-4: fp8e4 DoubleRow matmuls over integer levels - bit-exact,
    2x tensor-engine throughput.
  - Weight quantization (round(W/s)) done on device: ACT affine with +C
    round trick, DVE -C with dtype cast.
  - BN + QuantReLU epilogue fused: ACT per-feature affine, DVE round(+C,-C),
    DVE clip(min 15, max 0) with fp8 output cast.
  - Weight staging pools for layers 2-4 are shared and long-lived so the
    next layer's weight DMA + quant prefetches during the current layer.
"""

import numpy as np
import ml_dtypes  # noqa: F401

B, D_IN, H, C_OUT = 8192, 2048, 4096, 1000
NCORES = 8
BC = B // NCORES            # 1024 batch rows per core
N4P = 1024                  # padded final output feature dim (1000 -> 1024)
C_ROUND = float(1.5 * 2 ** 23)
EPS = 1e-5

_CACHE = {}


def _build_nc():
    import concourse.bass as bass  # noqa: F401
    from concourse import bacc
    import concourse.mybir as mybir
    import concourse.tile as tile

    dt = mybir.dt
    P = 128
    AF = mybir.ActivationFunctionType
    ALU = mybir.AluOpType

    nc = bacc.Bacc("TRN2", target_bir_lowering=False)

    # ---- DRAM I/O ----
    xt_hi = nc.dram_tensor("xt_hi", [D_IN, BC], dt.float16, kind="ExternalInput")
    xt_lo = nc.dram_tensor("xt_lo", [D_IN, BC], dt.float16, kind="ExternalInput")
    w1t = nc.dram_tensor("w1t", [D_IN, H], dt.float32, kind="ExternalInput")
    w2t = nc.dram_tensor("w2t", [H, H], dt.float32, kind="ExternalInput")
    w3t = nc.dram_tensor("w3t", [H, H], dt.float32, kind="ExternalInput")
    w4t = nc.dram_tensor("w4t", [H, N4P], dt.float32, kind="ExternalInput")
    ab1 = nc.dram_tensor("ab1", [H, 2], dt.float32, kind="ExternalInput")
    ab2 = nc.dram_tensor("ab2", [H, 2], dt.float32, kind="ExternalInput")
    ab3 = nc.dram_tensor("ab3", [H, 2], dt.float32, kind="ExternalInput")
    ab4 = nc.dram_tensor("ab4", [N4P, 2], dt.float32, kind="ExternalInput")
    inv_sb = nc.dram_tensor("inv_sb", [P, 4], dt.float32, kind="ExternalInput")
    out_t = nc.dram_tensor("out_t", [N4P, BC], dt.float32, kind="ExternalOutput")

    with tile.TileContext(nc) as tc:
        ppool_cm = tc.tile_pool(name="psum", bufs=6, space="PSUM")
        ppool = ppool_cm.__enter__()
        const_cm = tc.tile_pool(name="const", bufs=1)
        cpool = const_cm.__enter__()

        invs = cpool.tile([P, 4], dt.float32, name="invs")
        nc.sync.dma_start(invs[:], inv_sb[:])
        cbias = cpool.tile([P, 1], dt.float32, name="cbias")
        nc.vector.memset(cbias[:], C_ROUND)

        # A1 (layer-1 output levels) - lives through L2
        apool12_cm = tc.tile_pool(name="acts12", bufs=1)
        apool12 = apool12_cm.__enter__()
        A1 = apool12.tile([P, H // P, BC], dt.float8e4, name="A1")

        # long-lived weight staging pools shared by layers 2-4 (enables
        # cross-layer prefetch of weight DMA + quantization)
        wf234_cm = tc.tile_pool(name="wf234", bufs=3)
        wf234 = wf234_cm.__enter__()
        qt234_cm = tc.tile_pool(name="qt234", bufs=4)
        qt234 = qt234_cm.__enter__()
        tmp_cm = tc.tile_pool(name="tmpp", bufs=3)
        tmp_pool = tmp_cm.__enter__()
        abt_cm = tc.tile_pool(name="abtp", bufs=1)
        abt_pool = abt_cm.__enter__()

        def layer(wt, ab, K, N, mode, a_in, a_out, inv_idx, wf_pool, qt_pool,
                  wq_dt, out_stage_pool=None):
            KT = K // P
            NT = N // P
            abt = abt_pool.tile([P, NT, 2], dt.float32, name=f"abt{inv_idx}",
                                tag=f"abt{inv_idx}")
            nc.sync.dma_start(abt[:], ab[:].rearrange("(nt p) two -> p nt two", p=P))

            for nt in range(NT):
                n0 = nt * P
                wf = wf_pool.tile([P, KT, P], dt.float32,
                                  name=f"wf{inv_idx}", tag="wf")
                qt = qt_pool.tile([P, KT, P], wq_dt,
                                  name=f"qt{inv_idx}", tag="qt")
                wsrc = wt[:, n0:n0 + P].rearrange("(kt p) n -> p kt n", p=P)
                nsplit = 4 if KT % 4 == 0 else 2
                step = KT // nsplit
                for h in range(nsplit):
                    sl = slice(h * step, (h + 1) * step)
                    nc.sync.dma_start(wf[:, sl, :], wsrc[:, sl, :])
                    # quantize: ACT computes round(W*inv_s) + C in place (fp32)
                    nc.scalar.activation(
                        wf[:, sl, :], wf[:, sl, :], AF.Identity,
                        bias=cbias[:], scale=invs[:, inv_idx:inv_idx + 1],
                    )
                    nc.vector.tensor_scalar(qt[:, sl, :], wf[:, sl, :],
                                            C_ROUND, None, ALU.subtract)

                for b0 in range(0, BC, 512):
                    psum = ppool.tile([P, 512], dt.float32, name="psum", tag="ps")
                    if mode == "hilo":
                        nparts = len(a_in)
                        for pi in range(nparts):
                            for kt in range(KT):
                                nc.tensor.matmul(
                                    psum[:], qt[:, kt, :],
                                    a_in[pi][:, kt, b0:b0 + 512],
                                    start=(pi == 0 and kt == 0),
                                    stop=(pi == nparts - 1 and kt == KT - 1))
                    else:
                        for kp in range(KT // 2):
                            nc.tensor.matmul(
                                psum[:], qt[:, 2 * kp:2 * kp + 2, :],
                                a_in[:, 2 * kp:2 * kp + 2, b0:b0 + 512],
                                start=(kp == 0), stop=(kp == KT // 2 - 1),
                                perf_mode=mybir.MatmulPerfMode.DoubleRow)

                    if a_out is not None:
                        tmp = tmp_pool.tile([P, 512], dt.float32, name="tmp", tag="tmp")
                        nc.scalar.activation(
                            tmp[:], psum[:], AF.Identity,
                            bias=abt[:, nt, 1:2], scale=abt[:, nt, 0:1])
                        nc.vector.tensor_scalar(tmp[:], tmp[:], C_ROUND, C_ROUND,
                                                ALU.add, ALU.subtract)
                        nc.vector.tensor_scalar(a_out[:, nt, b0:b0 + 512], tmp[:],
                                                15.0, 0.0, ALU.min, ALU.max)
                    else:
                        ost = out_stage_pool.tile([P, 512], dt.float32,
                                                  name="ost", tag="ost")
                        nc.scalar.activation(
                            ost[:], psum[:], AF.Identity,
                            bias=abt[:, nt, 1:2], scale=abt[:, nt, 0:1])
                        nc.sync.dma_start(out_t[n0:n0 + P, b0:b0 + 512], ost[:])

        # ---- layer 1 (fp16 hi/lo) ----
        xt_pool_cm = tc.tile_pool(name="xtp", bufs=1)
        xt_pool = xt_pool_cm.__enter__()
        xh = xt_pool.tile([P, D_IN // P, BC], dt.float16, name="xh")
        xl = xt_pool.tile([P, D_IN // P, BC], dt.float16, name="xl")
        xh_src = xt_hi[:].rearrange("(kt p) b -> p kt b", p=P)
        xl_src = xt_lo[:].rearrange("(kt p) b -> p kt b", p=P)
        for kt in range(D_IN // P):
            nc.sync.dma_start(xh[:, kt, :], xh_src[:, kt, :])
        for kt in range(D_IN // P):
            nc.sync.dma_start(xl[:, kt, :], xl_src[:, kt, :])

        w1f_cm = tc.tile_pool(name="w1f", bufs=2)
        w1f = w1f_cm.__enter__()
        q1_cm = tc.tile_pool(name="q1", bufs=2)
        q1 = q1_cm.__enter__()

        layer(w1t, ab1, D_IN, H, "hilo", (xh, xl), A1, 0, w1f, q1, dt.float16)

        q1_cm.__exit__(None, None, None)
        w1f_cm.__exit__(None, None, None)
        xt_pool_cm.__exit__(None, None, None)

        # ---- layers 2-4 (fp8 DoubleRow) ----
        apool23_cm = tc.tile_pool(name="acts23", bufs=1)
        apool23 = apool23_cm.__enter__()
        A2 = apool23.tile([P, H // P, BC], dt.float8e4, name="A2")
        layer(w2t, ab2, H, H, "dr", A1, A2, 1, wf234, qt234, dt.float8e4)

        A3 = apool23.tile([P, H // P, BC], dt.float8e4, name="A3")
        layer(w3t, ab3, H, H, "dr", A2, A3, 2, wf234, qt234, dt.float8e4)

        ost_cm = tc.tile_pool(name="ostp", bufs=3)
        ost_pool = ost_cm.__enter__()
        layer(w4t, ab4, H, N4P, "dr", A3, None, 3, wf234, qt234, dt.float8e4,
              out_stage_pool=ost_pool)

        ost_cm.__exit__(None, None, None)
        apool23_cm.__exit__(None, None, None)
        abt_cm.__exit__(None, None, None)
        tmp_cm.__exit__(None, None, None)
        qt234_cm.__exit__(None, None, None)
        wf234_cm.__exit__(None, None, None)
        apool12_cm.__exit__(None, None, None)
        const_cm.__exit__(None, None, None)
        ppool_cm.__exit__(None, None, None)

    nc.compile()
    return nc


def _host_prep(inputs):
    f32 = np.float32

    def wscale(W):
        # mimic reference: s = max(|W|) / 3.0 in fp32
        return f32(np.max(np.abs(W))) / f32(3.0)

    s_w = [wscale(inputs[k]) for k in ("W1", "W2", "W3", "W4")]
    s_a = [f32(inputs[k][0]) for k in ("s1", "s2", "s3")]

    # per-feature affine folds (fp64 then cast once to fp32)
    def fold(l, s_prev):
        g = inputs[f"g{l}"].astype(np.float64)
        be = inputs[f"be{l}"].astype(np.float64)
        m = inputs[f"m{l}"].astype(np.float64)
        v = inputs[f"v{l}"].astype(np.float64)
        b = inputs[f"b{l}"].astype(np.float64)
        inv = 1.0 / np.sqrt(v + EPS)
        sl = float(s_a[l - 1])
        alpha = (float(s_prev) * float(s_w[l - 1]) * g * inv) / sl
        beta = ((b - m) * inv * g + be) / sl
        return alpha.astype(f32), beta.astype(f32)

    a1, b1 = fold(1, 1.0)
    a2, b2 = fold(2, s_a[0])
    a3, b3 = fold(3, s_a[1])
    a4 = np.full(N4P, float(s_a[2]) * float(s_w[3]), dtype=f32)
    b4 = np.zeros(N4P, dtype=f32)
    b4[:C_OUT] = inputs["b4"]

    def abpack(a, b):
        return np.ascontiguousarray(np.stack([a, b], axis=1))

    w1t = np.ascontiguousarray(inputs["W1"].T)
    w2t = np.ascontiguousarray(inputs["W2"].T)
    w3t = np.ascontiguousarray(inputs["W3"].T)
    w4t = np.zeros((H, N4P), dtype=f32)
    w4t[:, :C_OUT] = inputs["W4"].T

    inv_sb = np.broadcast_to(
        np.array([1.0 / s for s in s_w], dtype=f32)[None, :], (128, 4)
    ).copy()

    xt = inputs["x"].T  # [D_IN, B] view
    shared = dict(
        w1t=w1t, w2t=w2t, w3t=w3t, w4t=w4t,
        ab1=abpack(a1, b1), ab2=abpack(a2, b2), ab3=abpack(a3, b3),
        ab4=abpack(a4, b4),
        inv_sb=inv_sb,
    )
    in_maps = []
    for c in range(NCORES):
        xs = np.ascontiguousarray(xt[:, c * BC:(c + 1) * BC], dtype=f32)
        xhi = xs.astype(np.float16)
        xlo = (xs - xhi.astype(f32)).astype(np.float16)
        m = dict(shared)
        m["xt_hi"] = xhi
        m["xt_lo"] = xlo
        in_maps.append(m)
    return in_maps


def kernel(**inputs):
    from concourse.bass_utils import run_bass_kernel_spmd

    if "nc" not in _CACHE:
        _CACHE["nc"] = _build_nc()
    nc = _CACHE["nc"]

    in_maps = _host_prep(inputs)
    res = run_bass_kernel_spmd(nc, in_maps, core_ids=list(range(NCORES)))

    out = np.empty((B, C_OUT), dtype=np.float32)
    for c in range(NCORES):
        out[c * BC:(c + 1) * BC, :] = res.results[c]["out_t"][:C_OUT, :].T
    return out


# revision 7
# speedup vs baseline: 1.0194x; 1.0194x over previous
"""TRN2 Bass kernel for the quantized 4-layer MLP (dense_mlp, 8 cores).

Strategy:
  - Data-parallel over batch: each of the 8 cores gets 1024 of 8192 rows.
  - Activations kept transposed [feature, batch] on-chip the whole way;
    quantized activation LEVELS (ints 0..15) stored as fp8e4 (exact).
  - Layer 1: x split into fp16 hi+lo parts (2 fp16 matmuls; 22+ mantissa
    bits total => fp32-level accuracy), weights quantized to ints -3..3
    (exact in fp16).
  - Layers 2-4: fp8e4 DoubleRow matmuls over integer levels - bit-exact,
    2x tensor-engine throughput.
  - Weight quantization (round(W/s)) done on device: ACT affine with +C
    round trick, DVE -C with dtype cast.
  - BN + QuantReLU epilogue fused: ACT per-feature affine, DVE round(+C,-C),
    DVE clip(min 15, max 0) with fp8 output cast.
  - Weight staging pools for layers 2-4 are shared and long-lived so the
    next layer's weight DMA + quant prefetches during the current layer.
"""

import numpy as np
import ml_dtypes  # noqa: F401

B, D_IN, H, C_OUT = 8192, 2048, 4096, 1000
NCORES = 8
BC = B // NCORES            # 1024 batch rows per core
N4P = 1024                  # padded final output feature dim (1000 -> 1024)
C_ROUND = float(1.5 * 2 ** 23)
EPS = 1e-5

_CACHE = {}


def _build_nc():
    import concourse.bass as bass  # noqa: F401
    from concourse import bacc
    import concourse.mybir as mybir
    import concourse.tile as tile

    dt = mybir.dt
    P = 128
    AF = mybir.ActivationFunctionType
    ALU = mybir.AluOpType

    nc = bacc.Bacc("TRN2", target_bir_lowering=False)

    # ---- DRAM I/O ----
    xt_hi = nc.dram_tensor("xt_hi", [D_IN, BC], dt.float16, kind="ExternalInput")
    xt_lo = nc.dram_tensor("xt_lo", [D_IN, BC], dt.float16, kind="ExternalInput")
    w1t = nc.dram_tensor("w1t", [D_IN, H], dt.float32, kind="ExternalInput")
    w2t = nc.dram_tensor("w2t", [H, H], dt.float32, kind="ExternalInput")
    w3t = nc.dram_tensor("w3t", [H, H], dt.float32, kind="ExternalInput")
    w4t = nc.dram_tensor("w4t", [H, N4P], dt.float32, kind="ExternalInput")
    ab1 = nc.dram_tensor("ab1", [H, 2], dt.float32, kind="ExternalInput")
    ab2 = nc.dram_tensor("ab2", [H, 2], dt.float32, kind="ExternalInput")
    ab3 = nc.dram_tensor("ab3", [H, 2], dt.float32, kind="ExternalInput")
    ab4 = nc.dram_tensor("ab4", [N4P, 2], dt.float32, kind="ExternalInput")
    inv_sb = nc.dram_tensor("inv_sb", [P, 4], dt.float32, kind="ExternalInput")
    out_t = nc.dram_tensor("out_t", [N4P, BC], dt.float32, kind="ExternalOutput")

    with tile.TileContext(nc) as tc:
        ppool_cm = tc.tile_pool(name="psum", bufs=6, space="PSUM")
        ppool = ppool_cm.__enter__()
        const_cm = tc.tile_pool(name="const", bufs=1)
        cpool = const_cm.__enter__()

        invs = cpool.tile([P, 4], dt.float32, name="invs")
        nc.gpsimd.dma_start(invs[:], inv_sb[:])
        cbias = cpool.tile([P, 1], dt.float32, name="cbias")
        nc.vector.memset(cbias[:], C_ROUND)

        # A1 (layer-1 output levels) - lives through L2
        apool12_cm = tc.tile_pool(name="acts12", bufs=1)
        apool12 = apool12_cm.__enter__()
        A1 = apool12.tile([P, H // P, BC], dt.float8e4, name="A1")

        # long-lived weight staging pools shared by layers 2-4 (enables
        # cross-layer prefetch of weight DMA + quantization)
        wf234_cm = tc.tile_pool(name="wf234", bufs=3)
        wf234 = wf234_cm.__enter__()
        qt234_cm = tc.tile_pool(name="qt234", bufs=4)
        qt234 = qt234_cm.__enter__()
        tmp_cm = tc.tile_pool(name="tmpp", bufs=3)
        tmp_pool = tmp_cm.__enter__()
        abt_cm = tc.tile_pool(name="abtp", bufs=1)
        abt_pool = abt_cm.__enter__()

        def layer(wt, ab, K, N, mode, a_in, a_out, inv_idx, wf_pool, qt_pool,
                  wq_dt, out_stage_pool=None):
            KT = K // P
            NT = N // P
            abt = abt_pool.tile([P, NT, 2], dt.float32, name=f"abt{inv_idx}",
                                tag=f"abt{inv_idx}")
            nc.gpsimd.dma_start(abt[:], ab[:].rearrange("(nt p) two -> p nt two", p=P))

            for nt in range(NT):
                n0 = nt * P
                wf = wf_pool.tile([P, KT, P], dt.float32,
                                  name=f"wf{inv_idx}", tag="wf")
                qt = qt_pool.tile([P, KT, P], wq_dt,
                                  name=f"qt{inv_idx}", tag="qt")
                wsrc = wt[:, n0:n0 + P].rearrange("(kt p) n -> p kt n", p=P)
                nsplit = 2
                step = KT // nsplit
                for h in range(nsplit):
                    sl = slice(h * step, (h + 1) * step)
                    nc.sync.dma_start(wf[:, sl, :], wsrc[:, sl, :])
                    # quantize: ACT computes round(W*inv_s) + C in place (fp32)
                    nc.scalar.activation(
                        wf[:, sl, :], wf[:, sl, :], AF.Identity,
                        bias=cbias[:], scale=invs[:, inv_idx:inv_idx + 1],
                    )
                    nc.vector.tensor_scalar(qt[:, sl, :], wf[:, sl, :],
                                            C_ROUND, None, ALU.subtract)

                for b0 in range(0, BC, 512):
                    psum = ppool.tile([P, 512], dt.float32, name="psum", tag="ps")
                    if mode == "hilo":
                        nparts = len(a_in)
                        for pi in range(nparts):
                            for kt in range(KT):
                                nc.tensor.matmul(
                                    psum[:], qt[:, kt, :],
                                    a_in[pi][:, kt, b0:b0 + 512],
                                    start=(pi == 0 and kt == 0),
                                    stop=(pi == nparts - 1 and kt == KT - 1))
                    else:
                        for kp in range(KT // 2):
                            nc.tensor.matmul(
                                psum[:], qt[:, 2 * kp:2 * kp + 2, :],
                                a_in[:, 2 * kp:2 * kp + 2, b0:b0 + 512],
                                start=(kp == 0), stop=(kp == KT // 2 - 1),
                                perf_mode=mybir.MatmulPerfMode.DoubleRow)

                    if a_out is not None:
                        tmp = tmp_pool.tile([P, 512], dt.float32, name="tmp", tag="tmp")
                        nc.scalar.activation(
                            tmp[:], psum[:], AF.Identity,
                            bias=abt[:, nt, 1:2], scale=abt[:, nt, 0:1])
                        nc.vector.tensor_scalar(tmp[:], tmp[:], C_ROUND, C_ROUND,
                                                ALU.add, ALU.subtract)
                        nc.vector.tensor_scalar(a_out[:, nt, b0:b0 + 512], tmp[:],
                                                15.0, 0.0, ALU.min, ALU.max)
                    else:
                        ost = out_stage_pool.tile([P, 512], dt.float32,
                                                  name="ost", tag="ost")
                        nc.scalar.activation(
                            ost[:], psum[:], AF.Identity,
                            bias=abt[:, nt, 1:2], scale=abt[:, nt, 0:1])
                        nc.gpsimd.dma_start(out_t[n0:n0 + P, b0:b0 + 512], ost[:])

        # ---- layer 1 (fp16 hi/lo) ----
        xt_pool_cm = tc.tile_pool(name="xtp", bufs=1)
        xt_pool = xt_pool_cm.__enter__()
        xh = xt_pool.tile([P, D_IN // P, BC], dt.float16, name="xh")
        xl = xt_pool.tile([P, D_IN // P, BC], dt.float16, name="xl")
        xh_src = xt_hi[:].rearrange("(kt p) b -> p kt b", p=P)
        xl_src = xt_lo[:].rearrange("(kt p) b -> p kt b", p=P)
        KTX = D_IN // P
        for c0 in range(0, KTX, 4):
            nc.gpsimd.dma_start(xh[:, c0:c0 + 4, :], xh_src[:, c0:c0 + 4, :])
            nc.gpsimd.dma_start(xl[:, c0:c0 + 4, :], xl_src[:, c0:c0 + 4, :])

        w1f_cm = tc.tile_pool(name="w1f", bufs=2)
        w1f = w1f_cm.__enter__()
        q1_cm = tc.tile_pool(name="q1", bufs=2)
        q1 = q1_cm.__enter__()

        layer(w1t, ab1, D_IN, H, "hilo", (xh, xl), A1, 0, w1f, q1, dt.float16)

        q1_cm.__exit__(None, None, None)
        w1f_cm.__exit__(None, None, None)
        xt_pool_cm.__exit__(None, None, None)

        # ---- layers 2-4 (fp8 DoubleRow) ----
        apool23_cm = tc.tile_pool(name="acts23", bufs=1)
        apool23 = apool23_cm.__enter__()
        A2 = apool23.tile([P, H // P, BC], dt.float8e4, name="A2")
        layer(w2t, ab2, H, H, "dr", A1, A2, 1, wf234, qt234, dt.float8e4)

        A3 = apool23.tile([P, H // P, BC], dt.float8e4, name="A3")
        layer(w3t, ab3, H, H, "dr", A2, A3, 2, wf234, qt234, dt.float8e4)

        ost_cm = tc.tile_pool(name="ostp", bufs=3)
        ost_pool = ost_cm.__enter__()
        layer(w4t, ab4, H, N4P, "dr", A3, None, 3, wf234, qt234, dt.float8e4,
              out_stage_pool=ost_pool)

        ost_cm.__exit__(None, None, None)
        apool23_cm.__exit__(None, None, None)
        abt_cm.__exit__(None, None, None)
        tmp_cm.__exit__(None, None, None)
        qt234_cm.__exit__(None, None, None)
        wf234_cm.__exit__(None, None, None)
        apool12_cm.__exit__(None, None, None)
        const_cm.__exit__(None, None, None)
        ppool_cm.__exit__(None, None, None)

    nc.compile()
    return nc


def _host_prep(inputs):
    f32 = np.float32

    def wscale(W):
        # mimic reference: s = max(|W|) / 3.0 in fp32
        return f32(np.max(np.abs(W))) / f32(3.0)

    s_w = [wscale(inputs[k]) for k in ("W1", "W2", "W3", "W4")]
    s_a = [f32(inputs[k][0]) for k in ("s1", "s2", "s3")]

    # per-feature affine folds (fp64 then cast once to fp32)
    def fold(l, s_prev):
        g = inputs[f"g{l}"].astype(np.float64)
        be = inputs[f"be{l}"].astype(np.float64)
        m = inputs[f"m{l}"].astype(np.float64)
        v = inputs[f"v{l}"].astype(np.float64)
        b = inputs[f"b{l}"].astype(np.float64)
        inv = 1.0 / np.sqrt(v + EPS)
        sl = float(s_a[l - 1])
        alpha = (float(s_prev) * float(s_w[l - 1]) * g * inv) / sl
        beta = ((b - m) * inv * g + be) / sl
        return alpha.astype(f32), beta.astype(f32)

    a1, b1 = fold(1, 1.0)
    a2, b2 = fold(2, s_a[0])
    a3, b3 = fold(3, s_a[1])
    a4 = np.full(N4P, float(s_a[2]) * float(s_w[3]), dtype=f32)
    b4 = np.zeros(N4P, dtype=f32)
    b4[:C_OUT] = inputs["b4"]

    def abpack(a, b):
        return np.ascontiguousarray(np.stack([a, b], axis=1))

    w1t = np.ascontiguousarray(inputs["W1"].T)
    w2t = np.ascontiguousarray(inputs["W2"].T)
    w3t = np.ascontiguousarray(inputs["W3"].T)
    w4t = np.zeros((H, N4P), dtype=f32)
    w4t[:, :C_OUT] = inputs["W4"].T

    inv_sb = np.broadcast_to(
        np.array([1.0 / s for s in s_w], dtype=f32)[None, :], (128, 4)
    ).copy()

    xt = inputs["x"].T  # [D_IN, B] view
    shared = dict(
        w1t=w1t, w2t=w2t, w3t=w3t, w4t=w4t,
        ab1=abpack(a1, b1), ab2=abpack(a2, b2), ab3=abpack(a3, b3),
        ab4=abpack(a4, b4),
        inv_sb=inv_sb,
    )
    in_maps = []
    for c in range(NCORES):
        xs = np.ascontiguousarray(xt[:, c * BC:(c + 1) * BC], dtype=f32)
        xhi = xs.astype(np.float16)
        xlo = (xs - xhi.astype(f32)).astype(np.float16)
        m = dict(shared)
        m["xt_hi"] = xhi
        m["xt_lo"] = xlo
        in_maps.append(m)
    return in_maps


def kernel(**inputs):
    from concourse.bass_utils import run_bass_kernel_spmd

    if "nc" not in _CACHE:
        _CACHE["nc"] = _build_nc()
    nc = _CACHE["nc"]

    in_maps = _host_prep(inputs)
    res = run_bass_kernel_spmd(nc, in_maps, core_ids=list(range(NCORES)))

    out = np.empty((B, C_OUT), dtype=np.float32)
    for c in range(NCORES):
        out[c * BC:(c + 1) * BC, :] = res.results[c]["out_t"][:C_OUT, :].T
    return out
